# revision 3
# baseline (speedup 1.0000x reference)
"""Multi-head self-attention (B=4, T=2048, D=1024, H=16) on 8 TRN2 NeuronCores.

Reference quirk: softmax normalizes over the QUERY axis (dim=2 of
[B,H,T1,T2]), i.e. attn[q,k] = exp(s[q,k]) / sum_q' exp(s[q',k]).

Sharding (fully SPMD, one NEFF for all 8 cores):
  core c -> batch b = c//2, head-group g = c%2 (8 heads = 512 cols of Wq/Wk/Wv).
  Host pre-slices AND pre-transposes per-core inputs (xT, wqT/wkT/wvT), runs
  the kernel, and stitches the 8 transposed [512, T] output shards back
  together (host-side transpose: device emits outT, avoiding PE transposes).

Device algorithm per core (v3 — software-pipelined, dense-PE schedule):
  1. x is DMAed as 32 [128,512] quarter-tiles (t-major order) so pair-0
     QT/KT projection can start ~6us in, overlapping the DMA tail.
     Prologue: QT/KT for pairs 0 AND 1 (PE work hidden under the x DMA),
     V[0:4]. Remaining V tiles dribble through pair 0's chunk stream;
     QT/KT of pair p+1 dribble through pair p's stream (p>=1).
  2. Per head-pair, per 128-wide key chunk:
       S = K @ Q^T [128 k, 1024 q] per (head, q-half) in PSUM; the two
       heads' score MMs are interleaved at adjacent tile_position row
       groups (0 / 64) so the PE can stream them concurrently,
       P = exp(SCALE * S) via ScalarE PSUM->SBUF (bf16),
       Z[k] row-sums via DVE tensor_reduce over P (keeps ScalarE lean),
       V'[k,:] = V[k,:] / Z[k] into persistent zero-padded vpad tiles,
       outT[d, q] += vpad^T @ P accumulated over 16 chunks in PSUM.
     Emission is pipelined: scores for chunk c+1 are issued between the
     exp and AV of chunk c so neither PE nor ACT queues behind the other.
  3. Epilogue per pair: acc -> SBUF copy -> DMA to outT rows (no transpose).
"""

import numpy as np

B, T, D, H = 4, 2048, 1024, 16
DH = D // H
SCALE = 1.0 / (DH**0.5)
N_CORES = 8
E = D // 2  # 512 output cols per core (8 heads)
N_PAIRS = 4  # head-pairs per core
N_DC = D // 128  # 8 contraction chunks for projections
N_KC = T // 128  # 16 key chunks
N_TQ = 4  # x quarter-tiles along t
QB = 1024  # exp free-dim block (2 PSUM banks)
V_PRE = 4  # V tiles projected in the prologue; rest dribbled

_built = None  # (nc,) cache so repeat kernel() calls skip rebuild/recompile


def _np_reference(x, padding_mask, Wq, Wk, Wv):
    """Pure-numpy fallback, used only if the mask is not all-ones."""
    x64 = x.astype(np.float64)
    Q = (x64 @ Wq.T.astype(np.float64)).reshape(B, T, H, DH).transpose(0, 2, 1, 3)
    K = (x64 @ Wk.T.astype(np.float64)).reshape(B, T, H, DH).transpose(0, 2, 1, 3)
    V = (x64 @ Wv.T.astype(np.float64)).reshape(B, T, H, DH).transpose(0, 2, 1, 3)
    s = np.einsum("bhqd,bhkd->bhqk", Q, K) * SCALE
    s = np.where(padding_mask[:, None, :, :] == 0, -np.inf, s)
    s = s - s.max(axis=2, keepdims=True)
    p = np.exp(s)
    p = p / p.sum(axis=2, keepdims=True)
    out = np.einsum("bhqk,bhkd->bhqd", p, V)
    return out.transpose(0, 2, 1, 3).reshape(B, T, D).astype(np.float32)


def _split_multi_waits(nc):
    """Walrus caps sync waits at 1 per instruction; Tile's tail drain can carry
    several. Move the extras onto single-wait drains appended to the previous
    basic block (same engine, earlier in program order)."""
    import concourse.mybir as mybir

    blocks = list(nc.m.functions[0].blocks)
    for bi, blk in enumerate(blocks):
        for inst in blk.instructions:
            if type(inst).__name__ not in ("InstDrain", "InstNoOp", "InstEventSemaphore"):
                continue
            si = inst.sync_info
            if si is not None and si.on_wait and len(si.on_wait) > 1:
                waits = list(si.on_wait)
                keep, extra = waits[-1], waits[:-1]
                assert all(w.wait_mode == "sem-ge-imm" for w in extra), extra
                si.on_wait = [keep]
                assert bi > 0, "multi-wait in first block"
                prev = blocks[bi - 1]
                for j, w in enumerate(extra):
                    d = mybir.InstDrain(
                        name=f"{inst.name}-ws{j}",
                        engine=inst.engine,
                        sync_info=mybir.SyncInfo(on_wait=[w], on_update=[]),
                    )
                    prev.add_instruction(d)


def _build_kernel(tc, xT, wqT, wkT, wvT, outT):
    import concourse.bass as bass  # noqa: F401
    import concourse.mybir as mybir

    nc = tc.nc
    FP = mybir.dt.float32
    FR = mybir.dt.float32r
    BF = mybir.dt.bfloat16
    Exp = mybir.ActivationFunctionType.Exp
    AX = mybir.AxisListType.X
    ADD = mybir.AluOpType.add

    # long-lived pools
    singles = tc.alloc_tile_pool(name="singles", bufs=1)
    xw = tc.alloc_tile_pool(name="xw", bufs=1)
    wp = tc.alloc_tile_pool(name="wp", bufs=3)
    qkv = tc.alloc_tile_pool(name="qkv", bufs=1)
    sps = tc.alloc_tile_pool(name="sps", bufs=2, space="PSUM")
    accps = tc.alloc_tile_pool(name="accps", bufs=1, space="PSUM")
    pp = tc.alloc_tile_pool(name="pp", bufs=8)
    zp = tc.alloc_tile_pool(name="zp", bufs=4)
    op = tc.alloc_tile_pool(name="op", bufs=2)

    # ---- loads: wq/wk first (pair-0/1 projections), x quarters t-major ----
    def load_w(wap, label):
        ws = []
        for dc in range(N_DC):
            t = wp.tile([128, E], FR, name=f"{label}{dc}", tag=f"w{dc}")
            nc.sync.dma_start(out=t, in_=wap[dc * 128 : (dc + 1) * 128, :])
            ws.append(t)
        return ws

    wq = load_w(wqT, "wq")
    wk = load_w(wkT, "wk")

    xq = [[None] * N_TQ for _ in range(N_DC)]
    for tq in range(N_TQ):
        for dc in range(N_DC):
            t = xw.tile([128, 512], FR, name=f"x{dc}_{tq}", tag=f"x{dc}_{tq}")
            nc.sync.dma_start(
                out=t, in_=xT[dc * 128 : (dc + 1) * 128, tq * 512 : (tq + 1) * 512]
            )
            xq[dc][tq] = t
        if tq == 0:
            wv = load_w(wvT, "wv")

    # persistent zero-padded V' tiles: [parity][hi], data half written per chunk
    vpads = [[None, None], [None, None]]
    for par in range(2):
        for hi in range(2):
            vt = singles.tile([128, 128], BF, name=f"vp{par}{hi}")
            nc.gpsimd.memset(vt, 0.0)
            vpads[par][hi] = vt

    # ---- projection emitters (psum borrowed from the S pool tag) ----
    def project_eT_tile(ws, pair, tt, et):
        """One [128, 512] t-block of QT/KT pair tile `et` (bf16 [128, T])."""
        ps = sps.tile([128, QB], FP, name=f"ps_{et.tensor.name}_{tt}", tag="s")
        for dc in range(N_DC):
            nc.tensor.matmul(
                ps[:, 0:512],
                ws[dc][:, pair * 128 : (pair + 1) * 128],
                xq[dc][tt],
                start=(dc == 0),
                stop=(dc == N_DC - 1),
            )
        nc.vector.tensor_copy(et[:, tt * 512 : (tt + 1) * 512], ps[:, 0:512])

    def project_v_tile(tt):
        v = qkv.tile([128, E], BF, name=f"v{tt}", tag=f"v{tt}")
        ps = sps.tile([128, QB], FP, name=f"ps_v{tt}", tag="s")
        tq, to = divmod(tt, 4)
        for dc in range(N_DC):
            nc.tensor.matmul(
                ps[:, 0:512],
                xq[dc][tq][:, to * 128 : (to + 1) * 128],
                wv[dc],
                start=(dc == 0),
                stop=(dc == N_DC - 1),
            )
        nc.vector.tensor_copy(v, ps[:, 0:512])
        return v

    QT = [None] * N_PAIRS
    KT = [None] * N_PAIRS
    V = [None] * N_KC

    def alloc_pair(p):
        QT[p] = qkv.tile([128, T], BF, name=f"qt{p}", tag=f"qt{p}")
        KT[p] = qkv.tile([128, T], BF, name=f"kt{p}", tag=f"kt{p}")

    # prologue: QT/KT for pairs 0 and 1 (hidden under the x DMA), V[0:V_PRE].
    # tt-major so the earliest x quarters unlock work first.
    alloc_pair(0)
    alloc_pair(1)
    for tt in range(4):
        project_eT_tile(wq, 0, tt, QT[0])
        project_eT_tile(wk, 0, tt, KT[0])
    for tt in range(4):
        project_eT_tile(wq, 1, tt, QT[1])
        project_eT_tile(wk, 1, tt, KT[1])
    for tt in range(V_PRE):
        V[tt] = project_v_tile(tt)

    # dribble schedule: work[(p, c)] -> list of zero-arg emitters
    work = {}

    def add_work(p, c, fn):
        work.setdefault((p, c), []).append(fn)

    for c in range(N_KC - V_PRE):  # V[V_PRE..15] during pair 0, just-in-time
        add_work(0, c, (lambda tt: (lambda: V.__setitem__(tt, project_v_tile(tt))))(V_PRE + c))
    for p in range(1, N_PAIRS - 1):  # pair p+1 QT/KT: 8 blocks at c=4..11
        add_work(p, 3, (lambda q: (lambda: alloc_pair(q)))(p + 1))
        for tt in range(4):
            add_work(p, 4 + tt, (lambda q, t2: (lambda: project_eT_tile(wq, q, t2, QT[q])))(p + 1, tt))
        for tt in range(4):
            add_work(p, 8 + tt, (lambda q, t2: (lambda: project_eT_tile(wk, q, t2, KT[q])))(p + 1, tt))

    # ---- attention emitters ----
    def scores_qb(p, c, qb):
        """S tiles for one q-half, both heads interleaved at row groups 0/64
        so the two 64-contraction MM streams can overlap on the PE."""
        s = {}
        for hi in range(2):
            s[hi] = sps.tile([128, QB], FP, name=f"s_{p}_{c}_{hi}_{qb}", tag="s")
        for qt in range(2):
            q0 = qb * QB + qt * 512
            for hi, base in ((0, 0), (1, 64)):
                nc.tensor.matmul(
                    s[hi][:, qt * 512 : (qt + 1) * 512],
                    KT[p][base : base + 64, c * 128 : (c + 1) * 128],
                    QT[p][base : base + 64, q0 : q0 + 512],
                    start=True,
                    stop=True,
                    tile_position=(base, 0),
                )
        return s

    def exps_qb(p, c, qb, stiles, ptiles, zr):
        for hi in range(2):
            pt = pp.tile([128, QB], BF, name=f"p_{p}_{c}_{hi}_{qb}", tag="p")
            nc.scalar.activation(out=pt, in_=stiles[(hi, qb)], func=Exp, scale=SCALE)
            nc.vector.tensor_reduce(
                zr[:, 2 * hi + qb : 2 * hi + qb + 1], pt, axis=AX, op=ADD
            )
            ptiles[(hi, qb)] = pt

    def zchain(p, c, zr):
        za = zp.tile([128, 2], FP, name=f"za_{p}_{c}", tag="za")
        nc.vector.tensor_add(za[:, 0:1], zr[:, 0:1], zr[:, 1:2])
        nc.vector.tensor_add(za[:, 1:2], zr[:, 2:3], zr[:, 3:4])
        rz = zp.tile([128, 2], FP, name=f"rz_{p}_{c}", tag="rz")
        nc.vector.reciprocal(out=rz, in_=za)
        vt = vpads[c % 2]
        for hi in range(2):
            lo = hi * 64
            nc.vector.tensor_scalar_mul(
                vt[hi][:, lo : lo + 64],
                V[c][:, p * 128 + lo : p * 128 + lo + 64],
                rz[:, hi : hi + 1],
            )
        return vt

    def av_half(p, c, acc, vt, ptiles, hi):
        for qb in range(2):
            for qt in range(2):
                nc.tensor.matmul(
                    acc[qb][:, qt * 512 : (qt + 1) * 512],
                    vt[hi],
                    ptiles[(hi, qb)][:, qt * 512 : (qt + 1) * 512],
                    start=(c == 0 and hi == 0),
                    stop=(c == N_KC - 1 and hi == 1),
                )

    # ---- pipelined main loop ----
    stiles = {}
    for qb in range(2):
        for hi, s in scores_qb(0, 0, qb).items():
            stiles[(hi, qb)] = s

    for p in range(N_PAIRS):
        acc = [
            accps.tile([128, QB], FP, name=f"acc{qb}_{p}", tag=f"acc{qb}")
            for qb in range(2)
        ]
        for c in range(N_KC):
            zr = zp.tile([128, 4], FP, name=f"zr_{p}_{c}", tag="zr")
            ptiles = {}
            exps_qb(p, c, 0, stiles, ptiles, zr)
            exps_qb(p, c, 1, stiles, ptiles, zr)
            nxt = (p, c + 1) if c + 1 < N_KC else (p + 1, 0)
            vt = zchain(p, c, zr)
            stiles = {}
            if nxt[0] < N_PAIRS:
                for hi, s in scores_qb(*nxt, 0).items():
                    stiles[(hi, 0)] = s
            av_half(p, c, acc, vt, ptiles, 0)
            if nxt[0] < N_PAIRS:
                for hi, s in scores_qb(*nxt, 1).items():
                    stiles[(hi, 1)] = s
            av_half(p, c, acc, vt, ptiles, 1)
            for fn in work.get((p, c), []):
                fn()
        # epilogue: outT rows for this pair (host transposes back)
        for qb in range(2):
            ot = op.tile([128, QB], FP, name=f"ot_{p}_{qb}", tag="ot")
            nc.vector.tensor_copy(ot, acc[qb])
            nc.sync.dma_start(
                out=outT[p * 128 : (p + 1) * 128, qb * QB : (qb + 1) * QB], in_=ot
            )

    for pool in (op, zp, pp, accps, sps, qkv, wp, xw, singles):
        pool.release()


def build():
    import concourse.bacc as bacc
    import concourse.mybir as mybir
    import concourse.tile as tile

    nc = bacc.Bacc("TRN2", target_bir_lowering=False, debug=False)
    FP = mybir.dt.float32
    FR = mybir.dt.float32r
    xT = nc.dram_tensor("xT", [D, T], FR, kind="ExternalInput").ap()
    wqT = nc.dram_tensor("wqT", [D, E], FR, kind="ExternalInput").ap()
    wkT = nc.dram_tensor("wkT", [D, E], FR, kind="ExternalInput").ap()
    wvT = nc.dram_tensor("wvT", [D, E], FR, kind="ExternalInput").ap()
    outT = nc.dram_tensor("outT", [E, T], FP, kind="ExternalOutput").ap()
    with tile.TileContext(nc) as tc:
        _build_kernel(tc, xT, wqT, wkT, wvT, outT)
    nc.compile()
    _split_multi_waits(nc)
    return nc


def _get_nc():
    global _built
    if _built is None:
        _built = build()
    return _built


def make_in_maps(x, Wq, Wk, Wv):
    in_maps = []
    for c in range(N_CORES):
        b, g = divmod(c, 2)
        e0 = E * g
        in_maps.append(
            {
                "xT": np.ascontiguousarray(x[b].T),
                "wqT": np.ascontiguousarray(Wq[e0 : e0 + E, :].T),
                "wkT": np.ascontiguousarray(Wk[e0 : e0 + E, :].T),
                "wvT": np.ascontiguousarray(Wv[e0 : e0 + E, :].T),
            }
        )
    return in_maps


def assemble_out(results):
    out = np.empty((B, T, D), np.float32)
    for c in range(N_CORES):
        b, g = divmod(c, 2)
        e0 = E * g
        out[b][:, e0 : e0 + E] = results[c]["outT"].T
    return out


def kernel(x, padding_mask, Wq, Wk, Wv):
    x = np.asarray(x, dtype=np.float32)
    padding_mask = np.asarray(padding_mask, dtype=np.float32)
    Wq = np.asarray(Wq, dtype=np.float32)
    Wk = np.asarray(Wk, dtype=np.float32)
    Wv = np.asarray(Wv, dtype=np.float32)
    if not np.all(padding_mask == 1.0):
        return _np_reference(x, padding_mask, Wq, Wk, Wv)

    from concourse.bass_utils import run_bass_kernel_spmd

    nc = _get_nc()
    in_maps = make_in_maps(x, Wq, Wk, Wv)
    res = run_bass_kernel_spmd(nc, in_maps, list(range(N_CORES)))
    return assemble_out(res.results)


# revision 7
# speedup vs baseline: 1.0174x; 1.0174x over previous
"""Multi-head self-attention (B=4, T=2048, D=1024, H=16) on 8 TRN2 NeuronCores.

Reference quirk: softmax normalizes over the QUERY axis (dim=2 of
[B,H,T1,T2]), i.e. attn[q,k] = exp(s[q,k]) / sum_q' exp(s[q',k]).

Sharding (fully SPMD, one NEFF for all 8 cores):
  core c -> batch b = c//2, head-group g = c%2 (8 heads = 512 cols of Wq/Wk/Wv).
  Host pre-slices AND pre-transposes per-core inputs (xT, wqT/wkT/wvT), runs
  the kernel, and stitches the 8 transposed [512, T] output shards back
  together (host-side transpose: device emits outT, avoiding PE transposes).

Device algorithm per core (v3 — software-pipelined, dense-PE schedule):
  1. x is DMAed as 32 [128,512] quarter-tiles (t-major order) so pair-0
     QT/KT projection can start ~6us in, overlapping the DMA tail.
     Prologue: QT/KT for pairs 0 AND 1 (PE work hidden under the x DMA),
     V[0:4]. Remaining V tiles dribble through pair 0's chunk stream;
     QT/KT of pair p+1 dribble through pair p's stream (p>=1).
  2. Per head-pair, per 128-wide key chunk:
       S = K @ Q^T [128 k, 1024 q] per (head, q-half) in PSUM; the two
       heads' score MMs are interleaved at adjacent tile_position row
       groups (0 / 64) so the PE can stream them concurrently,
       P = exp(SCALE * S) via ScalarE PSUM->SBUF (bf16),
       Z[k] row-sums via DVE tensor_reduce over P (keeps ScalarE lean),
       V'[k,:] = V[k,:] / Z[k] into persistent zero-padded vpad tiles,
       outT[d, q] += vpad^T @ P accumulated over 16 chunks in PSUM.
     Emission is pipelined: scores for chunk c+1 are issued between the
     exp and AV of chunk c so neither PE nor ACT queues behind the other.
  3. Epilogue per pair: acc -> SBUF copy -> DMA to outT rows (no transpose).
"""

import numpy as np

B, T, D, H = 4, 2048, 1024, 16
DH = D // H
SCALE = 1.0 / (DH**0.5)
N_CORES = 8
E = D // 2  # 512 output cols per core (8 heads)
N_PAIRS = 4  # head-pairs per core
N_DC = D // 128  # 8 contraction chunks for projections
N_KC = T // 128  # 16 key chunks
N_TQ = 4  # x quarter-tiles along t
QB = 1024  # exp free-dim block (2 PSUM banks)
V_PRE = 4  # V tiles projected in the prologue; rest dribbled

_built = None  # (nc,) cache so repeat kernel() calls skip rebuild/recompile


def _np_reference(x, padding_mask, Wq, Wk, Wv):
    """Pure-numpy fallback, used only if the mask is not all-ones."""
    x64 = x.astype(np.float64)
    Q = (x64 @ Wq.T.astype(np.float64)).reshape(B, T, H, DH).transpose(0, 2, 1, 3)
    K = (x64 @ Wk.T.astype(np.float64)).reshape(B, T, H, DH).transpose(0, 2, 1, 3)
    V = (x64 @ Wv.T.astype(np.float64)).reshape(B, T, H, DH).transpose(0, 2, 1, 3)
    s = np.einsum("bhqd,bhkd->bhqk", Q, K) * SCALE
    s = np.where(padding_mask[:, None, :, :] == 0, -np.inf, s)
    s = s - s.max(axis=2, keepdims=True)
    p = np.exp(s)
    p = p / p.sum(axis=2, keepdims=True)
    out = np.einsum("bhqk,bhkd->bhqd", p, V)
    return out.transpose(0, 2, 1, 3).reshape(B, T, D).astype(np.float32)


def _split_multi_waits(nc):
    """Walrus caps sync waits at 1 per instruction; Tile's tail drain can carry
    several. Move the extras onto single-wait drains appended to the previous
    basic block (same engine, earlier in program order)."""
    import concourse.mybir as mybir

    blocks = list(nc.m.functions[0].blocks)
    for bi, blk in enumerate(blocks):
        for inst in blk.instructions:
            if type(inst).__name__ not in ("InstDrain", "InstNoOp", "InstEventSemaphore"):
                continue
            si = inst.sync_info
            if si is not None and si.on_wait and len(si.on_wait) > 1:
                waits = list(si.on_wait)
                keep, extra = waits[-1], waits[:-1]
                assert all(w.wait_mode == "sem-ge-imm" for w in extra), extra
                si.on_wait = [keep]
                assert bi > 0, "multi-wait in first block"
                prev = blocks[bi - 1]
                for j, w in enumerate(extra):
                    d = mybir.InstDrain(
                        name=f"{inst.name}-ws{j}",
                        engine=inst.engine,
                        sync_info=mybir.SyncInfo(on_wait=[w], on_update=[]),
                    )
                    prev.add_instruction(d)


def _build_kernel(tc, xT, wqT, wkT, wvT, outT):
    import concourse.bass as bass  # noqa: F401
    import concourse.mybir as mybir

    nc = tc.nc
    FP = mybir.dt.float32
    FR = mybir.dt.float32r
    BF = mybir.dt.bfloat16
    Exp = mybir.ActivationFunctionType.Exp
    AX = mybir.AxisListType.X
    ADD = mybir.AluOpType.add

    # long-lived pools
    singles = tc.alloc_tile_pool(name="singles", bufs=1)
    xw = tc.alloc_tile_pool(name="xw", bufs=1)
    wp = tc.alloc_tile_pool(name="wp", bufs=3)
    qkv = tc.alloc_tile_pool(name="qkv", bufs=1)
    sps = tc.alloc_tile_pool(name="sps", bufs=2, space="PSUM")
    accps = tc.alloc_tile_pool(name="accps", bufs=1, space="PSUM")
    pp = tc.alloc_tile_pool(name="pp", bufs=8)
    zp = tc.alloc_tile_pool(name="zp", bufs=4)
    op = tc.alloc_tile_pool(name="op", bufs=2)

    # ---- loads: wq/wk first (pair-0 projections), x quarters t-major ----
    def load_w(wap, label):
        ws = []
        for dc in range(N_DC):
            t = wp.tile([128, E], BF, name=f"{label}{dc}", tag=f"w{dc}")
            nc.sync.dma_start(out=t, in_=wap[dc * 128 : (dc + 1) * 128, :])
            ws.append(t)
        return ws

    wq = load_w(wqT, "wq")
    wk = load_w(wkT, "wk")

    xq = [[None] * N_TQ for _ in range(N_DC)]
    for tq in range(N_TQ):
        for dc in range(N_DC):
            t = xw.tile([128, 512], BF, name=f"x{dc}_{tq}", tag=f"x{dc}_{tq}")
            nc.sync.dma_start(
                out=t, in_=xT[dc * 128 : (dc + 1) * 128, tq * 512 : (tq + 1) * 512]
            )
            xq[dc][tq] = t
        if tq == 0:
            wv = load_w(wvT, "wv")

    # persistent zero-padded V' tiles: [parity][hi], data half written per chunk
    vpads = [[None, None], [None, None]]
    for par in range(2):
        for hi in range(2):
            vt = singles.tile([128, 128], BF, name=f"vp{par}{hi}")
            nc.gpsimd.memset(vt, 0.0)
            vpads[par][hi] = vt

    # ---- projection emitters (psum borrowed from the S pool tag) ----
    def project_eT_tile(ws, pair, tt, et):
        """One [128, 512] t-block of QT/KT pair tile `et` (bf16 [128, T])."""
        ps = sps.tile([128, QB], FP, name=f"ps_{et.tensor.name}_{tt}", tag="s")
        for dc in range(N_DC):
            nc.tensor.matmul(
                ps[:, 0:512],
                ws[dc][:, pair * 128 : (pair + 1) * 128],
                xq[dc][tt],
                start=(dc == 0),
                stop=(dc == N_DC - 1),
            )
        nc.vector.tensor_copy(et[:, tt * 512 : (tt + 1) * 512], ps[:, 0:512])

    def project_v_tile(tt):
        v = qkv.tile([128, E], BF, name=f"v{tt}", tag=f"v{tt}")
        ps = sps.tile([128, QB], FP, name=f"ps_v{tt}", tag="s")
        tq, to = divmod(tt, 4)
        for dc in range(N_DC):
            nc.tensor.matmul(
                ps[:, 0:512],
                xq[dc][tq][:, to * 128 : (to + 1) * 128],
                wv[dc],
                start=(dc == 0),
                stop=(dc == N_DC - 1),
            )
        nc.vector.tensor_copy(v, ps[:, 0:512])
        return v

    QT = [None] * N_PAIRS
    KT = [None] * N_PAIRS
    V = [None] * N_KC

    def alloc_pair(p):
        QT[p] = qkv.tile([128, T], BF, name=f"qt{p}", tag=f"qt{p}")
        KT[p] = qkv.tile([128, T], BF, name=f"kt{p}", tag=f"kt{p}")

    # prologue: pair-0 QT/KT (hidden under the x DMA), V[0:V_PRE].
    # tt-major so the earliest x quarters unlock work first.
    alloc_pair(0)
    for tt in range(4):
        project_eT_tile(wq, 0, tt, QT[0])
        project_eT_tile(wk, 0, tt, KT[0])
    for tt in range(V_PRE):
        V[tt] = project_v_tile(tt)

    # dribble schedule: work[(p, c)] -> list of zero-arg emitters
    work = {}

    def add_work(p, c, fn):
        work.setdefault((p, c), []).append(fn)

    for c in range(N_KC - V_PRE):  # V[V_PRE..15] during pair 0, just-in-time
        add_work(0, c, (lambda tt: (lambda: V.__setitem__(tt, project_v_tile(tt))))(V_PRE + c))
    for p in range(N_PAIRS - 1):  # pair p+1 QT/KT: 8 blocks at c=6..13
        add_work(p, 5, (lambda q: (lambda: alloc_pair(q)))(p + 1))
        for tt in range(4):
            add_work(p, 6 + tt, (lambda q, t2: (lambda: project_eT_tile(wq, q, t2, QT[q])))(p + 1, tt))
        for tt in range(4):
            add_work(p, 10 + tt, (lambda q, t2: (lambda: project_eT_tile(wk, q, t2, KT[q])))(p + 1, tt))

    # ---- attention emitters ----
    def scores_qb(p, c, qb):
        """S tiles for one q-half, both heads interleaved at row groups 0/64
        so the two 64-contraction MM streams can overlap on the PE."""
        s = {}
        for hi in range(2):
            s[hi] = sps.tile([128, QB], FP, name=f"s_{p}_{c}_{hi}_{qb}", tag="s")
        for qt in range(2):
            q0 = qb * QB + qt * 512
            for hi, base in ((0, 0), (1, 64)):
                nc.tensor.matmul(
                    s[hi][:, qt * 512 : (qt + 1) * 512],
                    KT[p][base : base + 64, c * 128 : (c + 1) * 128],
                    QT[p][base : base + 64, q0 : q0 + 512],
                    start=True,
                    stop=True,
                    tile_position=(base, 0),
                )
        return s

    def exps_qb(p, c, qb, stiles, ptiles, zr):
        for hi in range(2):
            pt = pp.tile([128, QB], BF, name=f"p_{p}_{c}_{hi}_{qb}", tag="p")
            nc.scalar.activation(out=pt, in_=stiles[(hi, qb)], func=Exp, scale=SCALE)
            nc.vector.tensor_reduce(
                zr[:, 2 * hi + qb : 2 * hi + qb + 1], pt, axis=AX, op=ADD
            )
            ptiles[(hi, qb)] = pt

    def zchain(p, c, zr):
        za = zp.tile([128, 2], FP, name=f"za_{p}_{c}", tag="za")
        nc.vector.tensor_add(za[:, 0:1], zr[:, 0:1], zr[:, 1:2])
        nc.vector.tensor_add(za[:, 1:2], zr[:, 2:3], zr[:, 3:4])
        rz = zp.tile([128, 2], FP, name=f"rz_{p}_{c}", tag="rz")
        nc.vector.reciprocal(out=rz, in_=za)
        vt = vpads[c % 2]
        for hi in range(2):
            lo = hi * 64
            nc.vector.tensor_scalar_mul(
                vt[hi][:, lo : lo + 64],
                V[c][:, p * 128 + lo : p * 128 + lo + 64],
                rz[:, hi : hi + 1],
            )
        return vt

    def av_half(p, c, acc, vt, ptiles, hi):
        for qb in range(2):
            for qt in range(2):
                nc.tensor.matmul(
                    acc[qb][:, qt * 512 : (qt + 1) * 512],
                    vt[hi],
                    ptiles[(hi, qb)][:, qt * 512 : (qt + 1) * 512],
                    start=(c == 0 and hi == 0),
                    stop=(c == N_KC - 1 and hi == 1),
                )

    # ---- pipelined main loop ----
    stiles = {}
    for qb in range(2):
        for hi, s in scores_qb(0, 0, qb).items():
            stiles[(hi, qb)] = s

    for p in range(N_PAIRS):
        acc = [
            accps.tile([128, QB], FP, name=f"acc{qb}_{p}", tag=f"acc{qb}")
            for qb in range(2)
        ]
        for c in range(N_KC):
            zr = zp.tile([128, 4], FP, name=f"zr_{p}_{c}", tag="zr")
            ptiles = {}
            exps_qb(p, c, 0, stiles, ptiles, zr)
            exps_qb(p, c, 1, stiles, ptiles, zr)
            nxt = (p, c + 1) if c + 1 < N_KC else (p + 1, 0)
            vt = zchain(p, c, zr)
            stiles = {}
            if nxt[0] < N_PAIRS:
                for hi, s in scores_qb(*nxt, 0).items():
                    stiles[(hi, 0)] = s
            av_half(p, c, acc, vt, ptiles, 0)
            if nxt[0] < N_PAIRS:
                for hi, s in scores_qb(*nxt, 1).items():
                    stiles[(hi, 1)] = s
            av_half(p, c, acc, vt, ptiles, 1)
            for fn in work.get((p, c), []):
                fn()
        # epilogue: outT rows for this pair (host transposes back)
        for qb in range(2):
            ot = op.tile([128, QB], FP, name=f"ot_{p}_{qb}", tag="ot")
            nc.vector.tensor_copy(ot, acc[qb])
            nc.sync.dma_start(
                out=outT[p * 128 : (p + 1) * 128, qb * QB : (qb + 1) * QB], in_=ot
            )

    for pool in (op, zp, pp, accps, sps, qkv, wp, xw, singles):
        pool.release()


def build():
    import concourse.bacc as bacc
    import concourse.mybir as mybir
    import concourse.tile as tile

    nc = bacc.Bacc("TRN2", target_bir_lowering=False, debug=False)
    FP = mybir.dt.float32
    BF = mybir.dt.bfloat16
    xT = nc.dram_tensor("xT", [D, T], BF, kind="ExternalInput").ap()
    wqT = nc.dram_tensor("wqT", [D, E], BF, kind="ExternalInput").ap()
    wkT = nc.dram_tensor("wkT", [D, E], BF, kind="ExternalInput").ap()
    wvT = nc.dram_tensor("wvT", [D, E], BF, kind="ExternalInput").ap()
    outT = nc.dram_tensor("outT", [E, T], FP, kind="ExternalOutput").ap()
    with tile.TileContext(nc) as tc:
        _build_kernel(tc, xT, wqT, wkT, wvT, outT)
    nc.compile()
    _split_multi_waits(nc)
    return nc


def _get_nc():
    global _built
    if _built is None:
        _built = build()
    return _built


def make_in_maps(x, Wq, Wk, Wv):
    import ml_dtypes

    bf16 = ml_dtypes.bfloat16
    in_maps = []
    for c in range(N_CORES):
        b, g = divmod(c, 2)
        e0 = E * g
        in_maps.append(
            {
                "xT": np.ascontiguousarray(x[b].T).astype(bf16),
                "wqT": np.ascontiguousarray(Wq[e0 : e0 + E, :].T).astype(bf16),
                "wkT": np.ascontiguousarray(Wk[e0 : e0 + E, :].T).astype(bf16),
                "wvT": np.ascontiguousarray(Wv[e0 : e0 + E, :].T).astype(bf16),
            }
        )
    return in_maps


def assemble_out(results):
    out = np.empty((B, T, D), np.float32)
    for c in range(N_CORES):
        b, g = divmod(c, 2)
        e0 = E * g
        out[b][:, e0 : e0 + E] = results[c]["outT"].T
    return out


def kernel(x, padding_mask, Wq, Wk, Wv):
    x = np.asarray(x, dtype=np.float32)
    padding_mask = np.asarray(padding_mask, dtype=np.float32)
    Wq = np.asarray(Wq, dtype=np.float32)
    Wk = np.asarray(Wk, dtype=np.float32)
    Wv = np.asarray(Wv, dtype=np.float32)
    if not np.all(padding_mask == 1.0):
        return _np_reference(x, padding_mask, Wq, Wk, Wv)

    from concourse.bass_utils import run_bass_kernel_spmd

    nc = _get_nc()
    in_maps = make_in_maps(x, Wq, Wk, Wv)
    res = run_bass_kernel_spmd(nc, in_maps, list(range(N_CORES)))
    return assemble_out(res.results)


# revision 10
# speedup vs baseline: 1.1329x; 1.1136x over previous
"""Multi-head self-attention (B=4, T=2048, D=1024, H=16) on 8 TRN2 NeuronCores.

Reference quirk: softmax normalizes over the QUERY axis (dim=2 of
[B,H,T1,T2]), i.e. attn[q,k] = exp(s[q,k]) / sum_q' exp(s[q',k]).

Sharding (fully SPMD, one NEFF for all 8 cores):
  core c -> batch b = c//2, head-group g = c%2 (8 heads = 512 cols of Wq/Wk/Wv).
  Host pre-slices AND pre-transposes per-core inputs (xT, wqT/wkT/wvT), runs
  the kernel, and stitches the 8 transposed [512, T] output shards back
  together (host-side transpose: device emits outT, avoiding PE transposes).

Device algorithm per core (v3 — software-pipelined, dense-PE schedule):
  1. x is DMAed as 32 [128,512] quarter-tiles (t-major order) so pair-0
     QT/KT projection can start ~6us in, overlapping the DMA tail.
     Prologue: QT/KT for pairs 0 AND 1 (PE work hidden under the x DMA),
     V[0:4]. Remaining V tiles dribble through pair 0's chunk stream;
     QT/KT of pair p+1 dribble through pair p's stream (p>=1).
  2. Per head-pair, per 128-wide key chunk:
       S = K @ Q^T [128 k, 1024 q] per (head, q-half) in PSUM; the two
       heads' score MMs are interleaved at adjacent tile_position row
       groups (0 / 64) so the PE can stream them concurrently,
       P = exp(SCALE * S) via ScalarE PSUM->SBUF (bf16),
       Z[k] row-sums via DVE tensor_reduce over P (keeps ScalarE lean),
       V'[k,:] = V[k,:] / Z[k] into persistent zero-padded vpad tiles,
       outT[d, q] += vpad^T @ P accumulated over 16 chunks in PSUM.
     Emission is pipelined: scores for chunk c+1 are issued between the
     exp and AV of chunk c so neither PE nor ACT queues behind the other.
  3. Epilogue per pair: acc -> SBUF copy -> DMA to outT rows (no transpose).
"""

import numpy as np

B, T, D, H = 4, 2048, 1024, 16
DH = D // H
SCALE = 1.0 / (DH**0.5)
N_CORES = 8
E = D // 2  # 512 output cols per core (8 heads)
N_PAIRS = 4  # head-pairs per core
N_DC = D // 128  # 8 contraction chunks for projections
N_KC = T // 128  # 16 key chunks
N_TQ = 4  # x quarter-tiles along t
QB = 1024  # exp free-dim block (2 PSUM banks)
V_PRE = 4  # V tiles projected in the prologue; rest dribbled

_built = None  # (nc,) cache so repeat kernel() calls skip rebuild/recompile


def _np_reference(x, padding_mask, Wq, Wk, Wv):
    """Pure-numpy fallback, used only if the mask is not all-ones."""
    x64 = x.astype(np.float64)
    Q = (x64 @ Wq.T.astype(np.float64)).reshape(B, T, H, DH).transpose(0, 2, 1, 3)
    K = (x64 @ Wk.T.astype(np.float64)).reshape(B, T, H, DH).transpose(0, 2, 1, 3)
    V = (x64 @ Wv.T.astype(np.float64)).reshape(B, T, H, DH).transpose(0, 2, 1, 3)
    s = np.einsum("bhqd,bhkd->bhqk", Q, K) * SCALE
    s = np.where(padding_mask[:, None, :, :] == 0, -np.inf, s)
    s = s - s.max(axis=2, keepdims=True)
    p = np.exp(s)
    p = p / p.sum(axis=2, keepdims=True)
    out = np.einsum("bhqk,bhkd->bhqd", p, V)
    return out.transpose(0, 2, 1, 3).reshape(B, T, D).astype(np.float32)


def _split_multi_waits(nc):
    """Walrus caps sync waits at 1 per instruction; Tile's tail drain can carry
    several. Move the extras onto single-wait drains appended to the previous
    basic block (same engine, earlier in program order)."""
    import concourse.mybir as mybir

    blocks = list(nc.m.functions[0].blocks)
    for bi, blk in enumerate(blocks):
        for inst in blk.instructions:
            if type(inst).__name__ not in ("InstDrain", "InstNoOp", "InstEventSemaphore"):
                continue
            si = inst.sync_info
            if si is not None and si.on_wait and len(si.on_wait) > 1:
                waits = list(si.on_wait)
                keep, extra = waits[-1], waits[:-1]
                assert all(w.wait_mode == "sem-ge-imm" for w in extra), extra
                si.on_wait = [keep]
                assert bi > 0, "multi-wait in first block"
                prev = blocks[bi - 1]
                for j, w in enumerate(extra):
                    d = mybir.InstDrain(
                        name=f"{inst.name}-ws{j}",
                        engine=inst.engine,
                        sync_info=mybir.SyncInfo(on_wait=[w], on_update=[]),
                    )
                    prev.add_instruction(d)


def _build_kernel(tc, xT, wqT, wkT, wvT, outT):
    import concourse.bass as bass  # noqa: F401
    import concourse.mybir as mybir

    nc = tc.nc
    FP = mybir.dt.float32
    FR = mybir.dt.float32r
    BF = mybir.dt.bfloat16
    Exp = mybir.ActivationFunctionType.Exp
    AX = mybir.AxisListType.X
    ADD = mybir.AluOpType.add

    # long-lived pools
    singles = tc.alloc_tile_pool(name="singles", bufs=1)
    xw = tc.alloc_tile_pool(name="xw", bufs=1)
    wp = tc.alloc_tile_pool(name="wp", bufs=3)
    qkv = tc.alloc_tile_pool(name="qkv", bufs=1)
    sps = tc.alloc_tile_pool(name="sps", bufs=2, space="PSUM")
    accps = tc.alloc_tile_pool(name="accps", bufs=1, space="PSUM")
    pp = tc.alloc_tile_pool(name="pp", bufs=8)
    zp = tc.alloc_tile_pool(name="zp", bufs=4)
    op = tc.alloc_tile_pool(name="op", bufs=2)

    # ---- loads: wq/wk first (pair-0 projections), x quarters t-major ----
    def load_w(wap, label):
        ws = []
        for dc in range(N_DC):
            t = wp.tile([128, E], BF, name=f"{label}{dc}", tag=f"w{dc}")
            nc.sync.dma_start(out=t, in_=wap[dc * 128 : (dc + 1) * 128, :])
            ws.append(t)
        return ws

    wq = load_w(wqT, "wq")

    xq = [[None] * N_TQ for _ in range(N_DC)]
    wk = wv = None
    for tq in range(N_TQ):
        for dc in range(N_DC):
            t = xw.tile([128, 512], BF, name=f"x{dc}_{tq}", tag=f"x{dc}_{tq}")
            nc.sync.dma_start(
                out=t, in_=xT[dc * 128 : (dc + 1) * 128, tq * 512 : (tq + 1) * 512]
            )
            xq[dc][tq] = t
        if tq == 0:
            wv = load_w(wvT, "wv")
            wk = load_w(wkT, "wk")

    # persistent zero-padded V' tiles: [parity][hi], data half written per chunk
    vpads = [[None, None], [None, None]]
    for par in range(2):
        for hi in range(2):
            vt = singles.tile([128, 128], BF, name=f"vp{par}{hi}")
            nc.gpsimd.memset(vt, 0.0)
            vpads[par][hi] = vt

    # ---- projection emitters (psum borrowed from the S pool tag) ----
    def project_eT_tile(ws, pair, tt, et):
        """One [128, 512] t-block of QT/KT pair tile `et` (bf16 [128, T])."""
        ps = sps.tile([128, QB], FP, name=f"ps_{et.tensor.name}_{tt}", tag="s")
        for dc in range(N_DC):
            nc.tensor.matmul(
                ps[:, 0:512],
                ws[dc][:, pair * 128 : (pair + 1) * 128],
                xq[dc][tt],
                start=(dc == 0),
                stop=(dc == N_DC - 1),
            )
        nc.vector.tensor_copy(et[:, tt * 512 : (tt + 1) * 512], ps[:, 0:512])

    def project_v_tile(tt):
        v = qkv.tile([128, E], BF, name=f"v{tt}", tag=f"v{tt}")
        ps = sps.tile([128, QB], FP, name=f"ps_v{tt}", tag="s")
        tq, to = divmod(tt, 4)
        for dc in range(N_DC):
            nc.tensor.matmul(
                ps[:, 0:512],
                xq[dc][tq][:, to * 128 : (to + 1) * 128],
                wv[dc],
                start=(dc == 0),
                stop=(dc == N_DC - 1),
            )
        nc.vector.tensor_copy(v, ps[:, 0:512])
        return v

    QT = [None] * N_PAIRS
    KT = [None] * N_PAIRS
    V = [None] * N_KC

    def alloc_pair(p):
        QT[p] = qkv.tile([128, T], BF, name=f"qt{p}", tag=f"qt{p}")
        KT[p] = qkv.tile([128, T], BF, name=f"kt{p}", tag=f"kt{p}")

    # prologue: pair-0 QT/KT (hidden under the x DMA), V[0:V_PRE].
    # Everything x-quarter-0 can feed comes first so the PE saturates while
    # the remaining x quarters stream in.
    alloc_pair(0)
    project_eT_tile(wq, 0, 0, QT[0])
    for tt in range(V_PRE):
        V[tt] = project_v_tile(tt)
    project_eT_tile(wk, 0, 0, KT[0])
    for tt in range(1, 4):
        project_eT_tile(wq, 0, tt, QT[0])
        project_eT_tile(wk, 0, tt, KT[0])

    # dribble schedule: work[(p, c)] -> list of zero-arg emitters
    work = {}

    def add_work(p, c, fn):
        work.setdefault((p, c), []).append(fn)

    for c in range(N_KC - V_PRE):  # V[V_PRE..15] during pair 0, just-in-time
        add_work(0, c, (lambda tt: (lambda: V.__setitem__(tt, project_v_tile(tt))))(V_PRE + c))
    for p in range(N_PAIRS - 1):  # pair p+1 QT/KT: 8 blocks at c=6..13
        add_work(p, 5, (lambda q: (lambda: alloc_pair(q)))(p + 1))
        for tt in range(4):
            add_work(p, 6 + tt, (lambda q, t2: (lambda: project_eT_tile(wq, q, t2, QT[q])))(p + 1, tt))
        for tt in range(4):
            add_work(p, 10 + tt, (lambda q, t2: (lambda: project_eT_tile(wk, q, t2, KT[q])))(p + 1, tt))

    # ---- attention emitters ----
    def scores_half(p, c, hi):
        """S tiles for one head of the pair: 2x [128k, 1024q] psum."""
        base = hi * 64
        out = {}
        for qb in range(2):
            s = sps.tile([128, QB], FP, name=f"s_{p}_{c}_{hi}_{qb}", tag="s")
            for qt in range(2):
                q0 = qb * QB + qt * 512
                nc.tensor.matmul(
                    s[:, qt * 512 : (qt + 1) * 512],
                    KT[p][base : base + 64, c * 128 : (c + 1) * 128],
                    QT[p][base : base + 64, q0 : q0 + 512],
                    start=True,
                    stop=True,
                    tile_position=(base, 0),
                )
            out[qb] = s
        return out

    def exps_half(p, c, hi, stiles, ptiles, zs):
        for qb in range(2):
            pt = pp.tile([128, QB], BF, name=f"p_{p}_{c}_{hi}_{qb}", tag="p")
            nc.scalar.activation(
                out=pt,
                in_=stiles[(hi, qb)],
                func=Exp,
                scale=SCALE,
                accum_out=zs[:, 2 * hi + qb : 2 * hi + qb + 1],
            )
            ptiles[(hi, qb)] = pt

    def zchain_half(p, c, hi, zs):
        """Z -> 1/Z -> scaled V' for one head; needs only that head's exps."""
        za = zp.tile([128, 1], FP, name=f"za_{p}_{c}_{hi}", tag=f"za{hi}")
        nc.vector.tensor_add(za, zs[:, 2 * hi : 2 * hi + 1], zs[:, 2 * hi + 1 : 2 * hi + 2])
        rz = zp.tile([128, 1], FP, name=f"rz_{p}_{c}_{hi}", tag=f"rz{hi}")
        nc.vector.reciprocal(out=rz, in_=za)
        vt = vpads[c % 2][hi]
        lo = hi * 64
        nc.vector.tensor_scalar_mul(
            vt[:, lo : lo + 64],
            V[c][:, p * 128 + lo : p * 128 + lo + 64],
            rz,
        )
        return vt

    def av_half(p, c, acc, vt, ptiles, hi):
        for qb in range(2):
            for qt in range(2):
                nc.tensor.matmul(
                    acc[qb][:, qt * 512 : (qt + 1) * 512],
                    vt,
                    ptiles[(hi, qb)][:, qt * 512 : (qt + 1) * 512],
                    start=(c == 0 and hi == 0),
                    stop=(c == N_KC - 1 and hi == 1),
                )

    # ---- pipelined main loop ----
    stiles = {}
    for hi in range(2):
        for qb, s in scores_half(0, 0, hi).items():
            stiles[(hi, qb)] = s

    for p in range(N_PAIRS):
        acc = [
            accps.tile([128, QB], FP, name=f"acc{qb}_{p}", tag=f"acc{qb}")
            for qb in range(2)
        ]
        for c in range(N_KC):
            zs = zp.tile([128, 4], FP, name=f"zs_{p}_{c}", tag="zs")
            ptiles = {}
            nxt = (p, c + 1) if c + 1 < N_KC else (p + 1, 0)
            # head 0: exps -> Z chain -> next-chunk scores -> AV
            exps_half(p, c, 0, stiles, ptiles, zs)
            vt0 = zchain_half(p, c, 0, zs)
            nstiles = {}
            if nxt[0] < N_PAIRS:
                for qb, s in scores_half(*nxt, 0).items():
                    nstiles[(0, qb)] = s
            av_half(p, c, acc, vt0, ptiles, 0)
            # head 1 likewise, overlapping head 0's AV with its exps
            exps_half(p, c, 1, stiles, ptiles, zs)
            vt1 = zchain_half(p, c, 1, zs)
            if nxt[0] < N_PAIRS:
                for qb, s in scores_half(*nxt, 1).items():
                    nstiles[(1, qb)] = s
            av_half(p, c, acc, vt1, ptiles, 1)
            stiles = nstiles
            for fn in work.get((p, c), []):
                fn()
        # epilogue: outT rows for this pair (host transposes back)
        for qb in range(2):
            ot = op.tile([128, QB], FP, name=f"ot_{p}_{qb}", tag="ot")
            nc.vector.tensor_copy(ot, acc[qb])
            nc.sync.dma_start(
                out=outT[p * 128 : (p + 1) * 128, qb * QB : (qb + 1) * QB], in_=ot
            )

    for pool in (op, zp, pp, accps, sps, qkv, wp, xw, singles):
        pool.release()


def build():
    import concourse.bacc as bacc
    import concourse.mybir as mybir
    import concourse.tile as tile

    nc = bacc.Bacc("TRN2", target_bir_lowering=False, debug=False)
    FP = mybir.dt.float32
    BF = mybir.dt.bfloat16
    xT = nc.dram_tensor("xT", [D, T], BF, kind="ExternalInput").ap()
    wqT = nc.dram_tensor("wqT", [D, E], BF, kind="ExternalInput").ap()
    wkT = nc.dram_tensor("wkT", [D, E], BF, kind="ExternalInput").ap()
    wvT = nc.dram_tensor("wvT", [D, E], BF, kind="ExternalInput").ap()
    outT = nc.dram_tensor("outT", [E, T], FP, kind="ExternalOutput").ap()
    with tile.TileContext(nc) as tc:
        _build_kernel(tc, xT, wqT, wkT, wvT, outT)
    nc.compile()
    _split_multi_waits(nc)
    return nc


def _get_nc():
    global _built
    if _built is None:
        _built = build()
    return _built


def make_in_maps(x, Wq, Wk, Wv):
    import ml_dtypes

    bf16 = ml_dtypes.bfloat16
    in_maps = []
    for c in range(N_CORES):
        b, g = divmod(c, 2)
        e0 = E * g
        in_maps.append(
            {
                "xT": np.ascontiguousarray(x[b].T).astype(bf16),
                "wqT": np.ascontiguousarray(Wq[e0 : e0 + E, :].T).astype(bf16),
                "wkT": np.ascontiguousarray(Wk[e0 : e0 + E, :].T).astype(bf16),
                "wvT": np.ascontiguousarray(Wv[e0 : e0 + E, :].T).astype(bf16),
            }
        )
    return in_maps


def assemble_out(results):
    out = np.empty((B, T, D), np.float32)
    for c in range(N_CORES):
        b, g = divmod(c, 2)
        e0 = E * g
        out[b][:, e0 : e0 + E] = results[c]["outT"].T
    return out


def kernel(x, padding_mask, Wq, Wk, Wv):
    x = np.asarray(x, dtype=np.float32)
    padding_mask = np.asarray(padding_mask, dtype=np.float32)
    Wq = np.asarray(Wq, dtype=np.float32)
    Wk = np.asarray(Wk, dtype=np.float32)
    Wv = np.asarray(Wv, dtype=np.float32)
    if not np.all(padding_mask == 1.0):
        return _np_reference(x, padding_mask, Wq, Wk, Wv)

    from concourse.bass_utils import run_bass_kernel_spmd

    nc = _get_nc()
    in_maps = make_in_maps(x, Wq, Wk, Wv)
    res = run_bass_kernel_spmd(nc, in_maps, list(range(N_CORES)))
    return assemble_out(res.results)


# revision 12
# speedup vs baseline: 1.1364x; 1.0031x over previous
"""Multi-head self-attention (B=4, T=2048, D=1024, H=16) on 8 TRN2 NeuronCores.

Reference quirk: softmax normalizes over the QUERY axis (dim=2 of
[B,H,T1,T2]), i.e. attn[q,k] = exp(s[q,k]) / sum_q' exp(s[q',k]).

Sharding (fully SPMD, one NEFF for all 8 cores):
  core c -> batch b = c//2, head-group g = c%2 (8 heads = 512 cols of Wq/Wk/Wv).
  Host pre-slices AND pre-transposes per-core inputs (xT, wqT/wkT/wvT), runs
  the kernel, and stitches the 8 transposed [512, T] output shards back
  together (host-side transpose: device emits outT, avoiding PE transposes).

Device algorithm per core (v3 — software-pipelined, dense-PE schedule):
  1. x is DMAed as 32 [128,512] quarter-tiles (t-major order) so pair-0
     QT/KT projection can start ~6us in, overlapping the DMA tail.
     Prologue: QT/KT for pairs 0 AND 1 (PE work hidden under the x DMA),
     V[0:4]. Remaining V tiles dribble through pair 0's chunk stream;
     QT/KT of pair p+1 dribble through pair p's stream (p>=1).
  2. Per head-pair, per 128-wide key chunk:
       S = K @ Q^T [128 k, 1024 q] per (head, q-half) in PSUM; the two
       heads' score MMs are interleaved at adjacent tile_position row
       groups (0 / 64) so the PE can stream them concurrently,
       P = exp(SCALE * S) via ScalarE PSUM->SBUF (bf16),
       Z[k] row-sums via DVE tensor_reduce over P (keeps ScalarE lean),
       V'[k,:] = V[k,:] / Z[k] into persistent zero-padded vpad tiles,
       outT[d, q] += vpad^T @ P accumulated over 16 chunks in PSUM.
     Emission is pipelined: scores for chunk c+1 are issued between the
     exp and AV of chunk c so neither PE nor ACT queues behind the other.
  3. Epilogue per pair: acc -> SBUF copy -> DMA to outT rows (no transpose).
"""

import numpy as np

B, T, D, H = 4, 2048, 1024, 16
DH = D // H
SCALE = 1.0 / (DH**0.5)
N_CORES = 8
E = D // 2  # 512 output cols per core (8 heads)
N_PAIRS = 4  # head-pairs per core
N_DC = D // 128  # 8 contraction chunks for projections
N_KC = T // 128  # 16 key chunks
N_TQ = 4  # x quarter-tiles along t
QB = 1024  # exp free-dim block (2 PSUM banks)
V_PRE = 8  # V tiles projected in the prologue; rest dribbled

_built = None  # (nc,) cache so repeat kernel() calls skip rebuild/recompile


def _np_reference(x, padding_mask, Wq, Wk, Wv):
    """Pure-numpy fallback, used only if the mask is not all-ones."""
    x64 = x.astype(np.float64)
    Q = (x64 @ Wq.T.astype(np.float64)).reshape(B, T, H, DH).transpose(0, 2, 1, 3)
    K = (x64 @ Wk.T.astype(np.float64)).reshape(B, T, H, DH).transpose(0, 2, 1, 3)
    V = (x64 @ Wv.T.astype(np.float64)).reshape(B, T, H, DH).transpose(0, 2, 1, 3)
    s = np.einsum("bhqd,bhkd->bhqk", Q, K) * SCALE
    s = np.where(padding_mask[:, None, :, :] == 0, -np.inf, s)
    s = s - s.max(axis=2, keepdims=True)
    p = np.exp(s)
    p = p / p.sum(axis=2, keepdims=True)
    out = np.einsum("bhqk,bhkd->bhqd", p, V)
    return out.transpose(0, 2, 1, 3).reshape(B, T, D).astype(np.float32)


def _split_multi_waits(nc):
    """Walrus caps sync waits at 1 per instruction; Tile's tail drain can carry
    several. Move the extras onto single-wait drains appended to the previous
    basic block (same engine, earlier in program order)."""
    import concourse.mybir as mybir

    blocks = list(nc.m.functions[0].blocks)
    for bi, blk in enumerate(blocks):
        for inst in blk.instructions:
            if type(inst).__name__ not in ("InstDrain", "InstNoOp", "InstEventSemaphore"):
                continue
            si = inst.sync_info
            if si is not None and si.on_wait and len(si.on_wait) > 1:
                waits = list(si.on_wait)
                keep, extra = waits[-1], waits[:-1]
                assert all(w.wait_mode == "sem-ge-imm" for w in extra), extra
                si.on_wait = [keep]
                assert bi > 0, "multi-wait in first block"
                prev = blocks[bi - 1]
                for j, w in enumerate(extra):
                    d = mybir.InstDrain(
                        name=f"{inst.name}-ws{j}",
                        engine=inst.engine,
                        sync_info=mybir.SyncInfo(on_wait=[w], on_update=[]),
                    )
                    prev.add_instruction(d)


def _build_kernel(tc, xT, wqT, wkT, wvT, outT):
    import concourse.bass as bass  # noqa: F401
    import concourse.mybir as mybir

    nc = tc.nc
    FP = mybir.dt.float32
    FR = mybir.dt.float32r
    BF = mybir.dt.bfloat16
    Exp = mybir.ActivationFunctionType.Exp
    AX = mybir.AxisListType.X
    ADD = mybir.AluOpType.add

    # long-lived pools
    singles = tc.alloc_tile_pool(name="singles", bufs=1)
    xw = tc.alloc_tile_pool(name="xw", bufs=1)
    wp = tc.alloc_tile_pool(name="wp", bufs=3)
    qkv = tc.alloc_tile_pool(name="qkv", bufs=1)
    sps = tc.alloc_tile_pool(name="sps", bufs=2, space="PSUM")
    accps = tc.alloc_tile_pool(name="accps", bufs=1, space="PSUM")
    pp = tc.alloc_tile_pool(name="pp", bufs=8)
    zp = tc.alloc_tile_pool(name="zp", bufs=4)
    op = tc.alloc_tile_pool(name="op", bufs=2)

    # ---- loads: wq/wk first (pair-0 projections), x quarters t-major ----
    def load_w(wap, label):
        ws = []
        for dc in range(N_DC):
            t = wp.tile([128, E], BF, name=f"{label}{dc}", tag=f"w{dc}")
            nc.sync.dma_start(out=t, in_=wap[dc * 128 : (dc + 1) * 128, :])
            ws.append(t)
        return ws

    wq = load_w(wqT, "wq")

    xq = [[None] * N_TQ for _ in range(N_DC)]
    wk = wv = None
    for tq in range(N_TQ):
        for dc in range(N_DC):
            t = xw.tile([128, 512], BF, name=f"x{dc}_{tq}", tag=f"x{dc}_{tq}")
            nc.sync.dma_start(
                out=t, in_=xT[dc * 128 : (dc + 1) * 128, tq * 512 : (tq + 1) * 512]
            )
            xq[dc][tq] = t
        if tq == 0:
            wv = load_w(wvT, "wv")
            wk = load_w(wkT, "wk")

    # persistent zero-padded V' tiles: [parity][hi], data half written per chunk
    vpads = [[None, None], [None, None]]
    for par in range(2):
        for hi in range(2):
            vt = singles.tile([128, 128], BF, name=f"vp{par}{hi}")
            nc.gpsimd.memset(vt, 0.0)
            vpads[par][hi] = vt

    # ---- projection emitters (psum borrowed from the S pool tag) ----
    def project_eT_tile(ws, pair, tt, et):
        """One [128, 512] t-block of QT/KT pair tile `et` (bf16 [128, T])."""
        ps = sps.tile([128, QB], FP, name=f"ps_{et.tensor.name}_{tt}", tag="s")
        for dc in range(N_DC):
            nc.tensor.matmul(
                ps[:, 0:512],
                ws[dc][:, pair * 128 : (pair + 1) * 128],
                xq[dc][tt],
                start=(dc == 0),
                stop=(dc == N_DC - 1),
            )
        nc.vector.tensor_copy(et[:, tt * 512 : (tt + 1) * 512], ps[:, 0:512])

    def project_v_tile(tt):
        v = qkv.tile([128, E], BF, name=f"v{tt}", tag=f"v{tt}")
        ps = sps.tile([128, QB], FP, name=f"ps_v{tt}", tag="s")
        tq, to = divmod(tt, 4)
        for dc in range(N_DC):
            nc.tensor.matmul(
                ps[:, 0:512],
                xq[dc][tq][:, to * 128 : (to + 1) * 128],
                wv[dc],
                start=(dc == 0),
                stop=(dc == N_DC - 1),
            )
        nc.vector.tensor_copy(v, ps[:, 0:512])
        return v

    QT = [None] * N_PAIRS
    KT = [None] * N_PAIRS
    V = [None] * N_KC

    def alloc_pair(p):
        QT[p] = qkv.tile([128, T], BF, name=f"qt{p}", tag=f"qt{p}")
        KT[p] = qkv.tile([128, T], BF, name=f"kt{p}", tag=f"kt{p}")

    # prologue: only what scores(0,0)/exp(0,0) truly gate on — all of QT0
    # (every chunk reads all queries), KT0's chunk-0 block, early V tiles.
    # KT0's later blocks (needed from chunk 4 on) dribble into pair 0.
    alloc_pair(0)
    project_eT_tile(wq, 0, 0, QT[0])
    project_eT_tile(wk, 0, 0, KT[0])
    for tt in range(V_PRE):
        V[tt] = project_v_tile(tt)
    for tt in range(1, 4):
        project_eT_tile(wq, 0, tt, QT[0])

    # dribble schedule: work[(p, c)] -> list of zero-arg emitters.
    # Per pair: own KT t1-3 at c=0..2, next pair's QT + KT-t0 at c=9..13,
    # plus (pair 0 only) the remaining V tiles just-in-time.
    work = {}

    def add_work(p, c, fn):
        work.setdefault((p, c), []).append(fn)

    def proj_item(ws, q, t2):
        ets = QT if ws is wq else KT
        return lambda: project_eT_tile(ws, q, t2, ets[q])

    for p in range(N_PAIRS):
        for tt in range(1, 4):  # own KT t1-3 (first used at chunk 4*tt)
            add_work(p, tt - 1, proj_item(wk, p, tt))
        if p < N_PAIRS - 1:
            add_work(p, 8, (lambda q: (lambda: alloc_pair(q)))(p + 1))
            for tt in range(4):
                add_work(p, 9 + tt, proj_item(wq, p + 1, tt))
            add_work(p, 13, proj_item(wk, p + 1, 0))
    for i in range(N_KC - V_PRE):  # V[V_PRE..15] during pair 0, just-in-time
        add_work(0, 3 + i, (lambda tt: (lambda: V.__setitem__(tt, project_v_tile(tt))))(V_PRE + i))

    # ---- attention emitters ----
    def scores_half(p, c, hi):
        """S tiles for one head of the pair: 2x [128k, 1024q] psum."""
        base = hi * 64
        out = {}
        for qb in range(2):
            s = sps.tile([128, QB], FP, name=f"s_{p}_{c}_{hi}_{qb}", tag="s")
            for qt in range(2):
                q0 = qb * QB + qt * 512
                nc.tensor.matmul(
                    s[:, qt * 512 : (qt + 1) * 512],
                    KT[p][base : base + 64, c * 128 : (c + 1) * 128],
                    QT[p][base : base + 64, q0 : q0 + 512],
                    start=True,
                    stop=True,
                    tile_position=(base, 0),
                )
            out[qb] = s
        return out

    def exps_half(p, c, hi, stiles, ptiles, zs):
        for qb in range(2):
            pt = pp.tile([128, QB], BF, name=f"p_{p}_{c}_{hi}_{qb}", tag="p")
            nc.scalar.activation(
                out=pt,
                in_=stiles[(hi, qb)],
                func=Exp,
                scale=SCALE,
                accum_out=zs[:, 2 * hi + qb : 2 * hi + qb + 1],
            )
            ptiles[(hi, qb)] = pt

    def zchain_half(p, c, hi, zs):
        """Z -> 1/Z -> scaled V' for one head; needs only that head's exps."""
        za = zp.tile([128, 1], FP, name=f"za_{p}_{c}_{hi}", tag=f"za{hi}")
        nc.vector.tensor_add(za, zs[:, 2 * hi : 2 * hi + 1], zs[:, 2 * hi + 1 : 2 * hi + 2])
        rz = zp.tile([128, 1], FP, name=f"rz_{p}_{c}_{hi}", tag=f"rz{hi}")
        nc.vector.reciprocal(out=rz, in_=za)
        vt = vpads[c % 2][hi]
        lo = hi * 64
        nc.vector.tensor_scalar_mul(
            vt[:, lo : lo + 64],
            V[c][:, p * 128 + lo : p * 128 + lo + 64],
            rz,
        )
        return vt

    def av_half(p, c, acc, vt, ptiles, hi):
        for qb in range(2):
            for qt in range(2):
                nc.tensor.matmul(
                    acc[qb][:, qt * 512 : (qt + 1) * 512],
                    vt,
                    ptiles[(hi, qb)][:, qt * 512 : (qt + 1) * 512],
                    start=(c == 0 and hi == 0),
                    stop=(c == N_KC - 1 and hi == 1),
                )

    # ---- pipelined main loop ----
    stiles = {}
    for hi in range(2):
        for qb, s in scores_half(0, 0, hi).items():
            stiles[(hi, qb)] = s

    for p in range(N_PAIRS):
        acc = [
            accps.tile([128, QB], FP, name=f"acc{qb}_{p}", tag=f"acc{qb}")
            for qb in range(2)
        ]
        for c in range(N_KC):
            zs = zp.tile([128, 4], FP, name=f"zs_{p}_{c}", tag="zs")
            ptiles = {}
            nxt = (p, c + 1) if c + 1 < N_KC else (p + 1, 0)
            # head 0: exps -> Z chain -> next-chunk scores -> AV
            exps_half(p, c, 0, stiles, ptiles, zs)
            vt0 = zchain_half(p, c, 0, zs)
            nstiles = {}
            if nxt[0] < N_PAIRS:
                for qb, s in scores_half(*nxt, 0).items():
                    nstiles[(0, qb)] = s
            av_half(p, c, acc, vt0, ptiles, 0)
            # head 1 likewise, overlapping head 0's AV with its exps
            exps_half(p, c, 1, stiles, ptiles, zs)
            vt1 = zchain_half(p, c, 1, zs)
            if nxt[0] < N_PAIRS:
                for qb, s in scores_half(*nxt, 1).items():
                    nstiles[(1, qb)] = s
            av_half(p, c, acc, vt1, ptiles, 1)
            stiles = nstiles
            for fn in work.get((p, c), []):
                fn()
        # epilogue: outT rows for this pair (host transposes back)
        for qb in range(2):
            ot = op.tile([128, QB], FP, name=f"ot_{p}_{qb}", tag="ot")
            nc.vector.tensor_copy(ot, acc[qb])
            nc.sync.dma_start(
                out=outT[p * 128 : (p + 1) * 128, qb * QB : (qb + 1) * QB], in_=ot
            )

    for pool in (op, zp, pp, accps, sps, qkv, wp, xw, singles):
        pool.release()


def build():
    import concourse.bacc as bacc
    import concourse.mybir as mybir
    import concourse.tile as tile

    nc = bacc.Bacc("TRN2", target_bir_lowering=False, debug=False)
    FP = mybir.dt.float32
    BF = mybir.dt.bfloat16
    xT = nc.dram_tensor("xT", [D, T], BF, kind="ExternalInput").ap()
    wqT = nc.dram_tensor("wqT", [D, E], BF, kind="ExternalInput").ap()
    wkT = nc.dram_tensor("wkT", [D, E], BF, kind="ExternalInput").ap()
    wvT = nc.dram_tensor("wvT", [D, E], BF, kind="ExternalInput").ap()
    outT = nc.dram_tensor("outT", [E, T], FP, kind="ExternalOutput").ap()
    with tile.TileContext(nc) as tc:
        _build_kernel(tc, xT, wqT, wkT, wvT, outT)
    nc.compile()
    _split_multi_waits(nc)
    return nc


def _get_nc():
    global _built
    if _built is None:
        _built = build()
    return _built


def make_in_maps(x, Wq, Wk, Wv):
    import ml_dtypes

    bf16 = ml_dtypes.bfloat16
    in_maps = []
    for c in range(N_CORES):
        b, g = divmod(c, 2)
        e0 = E * g
        in_maps.append(
            {
                "xT": np.ascontiguousarray(x[b].T).astype(bf16),
                "wqT": np.ascontiguousarray(Wq[e0 : e0 + E, :].T).astype(bf16),
                "wkT": np.ascontiguousarray(Wk[e0 : e0 + E, :].T).astype(bf16),
                "wvT": np.ascontiguousarray(Wv[e0 : e0 + E, :].T).astype(bf16),
            }
        )
    return in_maps


def assemble_out(results):
    out = np.empty((B, T, D), np.float32)
    for c in range(N_CORES):
        b, g = divmod(c, 2)
        e0 = E * g
        out[b][:, e0 : e0 + E] = results[c]["outT"].T
    return out


def kernel(x, padding_mask, Wq, Wk, Wv):
    x = np.asarray(x, dtype=np.float32)
    padding_mask = np.asarray(padding_mask, dtype=np.float32)
    Wq = np.asarray(Wq, dtype=np.float32)
    Wk = np.asarray(Wk, dtype=np.float32)
    Wv = np.asarray(Wv, dtype=np.float32)
    if not np.all(padding_mask == 1.0):
        return _np_reference(x, padding_mask, Wq, Wk, Wv)

    from concourse.bass_utils import run_bass_kernel_spmd

    nc = _get_nc()
    in_maps = make_in_maps(x, Wq, Wk, Wv)
    res = run_bass_kernel_spmd(nc, in_maps, list(range(N_CORES)))
    return assemble_out(res.results)
